# revision 54
# baseline (speedup 1.0000x reference)
"""Chunked-prefill paged attention kernel for Trainium2 (Bass/Tile), 8 cores.

Sharding: tensor-parallel over heads. Core i handles q heads 4i..4i+3 and
kv head i. The paged-cache scatter/gather (pure data movement, index-driven)
is resolved on the host; each core runs dense attention over the gathered
[ctx | chunk] keys/values for its kv head.

Per-core layout ("transposed scores"): q and k arrive pre-transposed and
pre-cast to fp16 from the host ([d, seq] / [d, L]):
  scoresT[l, q] = kT_tile (stationary) x qT (moving)     -> PSUM f32
  exp on the scalar engine (PSUM -> SBUF, fp16)
  oT[d, q]     += v_tile (stationary) x expT (moving)    -> PSUM f32
Tasks cover up to THREE 128-row l-tiles so each exp instruction amortizes
the fixed PSUM/SBUF access overhead over 1536 columns; the PSUM budget is
exactly 8 banks: score tiles [128,3,512] (3 banks) double-buffered + the
output accumulator (1 bank) double-buffered.

Softmax denominators: the DVE accumulates the fp16 exp tiles elementwise
(2-byte operands hit the DVE 2x mode) into 4 interleaved accumulator rows
of one [128, 4, 512] tile per group; the tile ships to the host as-is and
the host does the final partition-fold + divide (cheap numpy). The causal
mask is a 0/1 multiply on the exp output (DVE), off the ACT critical path.

fp16 operands stream the PE at 1 cycle/row (same as f32r) with no minimum
moving-width constraint, so causal trimming is exact at 128 granularity.
"""

import numpy as np

import concourse.bacc as bacc
import concourse.bass as bass
import concourse.mybir as mybir
import concourse.tile as tile
from concourse.bass_utils import run_bass_kernel_spmd

NH, NKVH, HD = 32, 8, 128
SCALE = 0.08838834764831845  # 1/sqrt(128)
SEQ, CTX = 1024, 3072
L = CTX + SEQ  # 4096
NDEV = 8
HPD = NH // NDEV  # q heads per device
QCH = 512  # q columns per group (psum bank width in f32)
NQC = SEQ // QCH  # q chunks
NDACC = 4  # interleaved fp16 denominator accumulator rows per group
NSLOT = 3  # l-tiles per task
NT = L // 128  # 32 l-tiles total
NT_CTX = CTX // 128  # 24 context l-tiles

F32 = mybir.dt.float32
F16 = mybir.dt.float16

# kdT arrives from the host with l-tiles permuted into consumption order
# [0..15 | 24..31 | 16..23], so every k DMA is a contiguous range. Four
# descriptors, sized so each arrives just before its first consumer.
K_SEGS = [(0, 3), (3, 9), (9, 24), (24, 32)]  # in sbuf tile positions
K_TILE_POS = {}  # dram l-tile -> sbuf tile position
for _lt in range(32):
    K_TILE_POS[_lt] = _lt if _lt < 16 else (_lt - 8 if _lt >= 24 else _lt + 8)

_CACHE = {}


def _group_tasks(h, c, final):
    """Slot lists for group (h, c): 8 context triples + chunk tasks.

    Context tiles fill triples; masked chunk tiles group together so the
    exp can skip their shared fully-masked left region. Chunk tasks sit
    mid-group (the DVE mask-mul rides behind PE work), except in the
    final group where they come last so the drain tail is the narrowest
    task.
    """
    ctx = [[3 * i, 3 * i + 1, 3 * i + 2] for i in range(NT_CTX // 3)]
    if c == 0:
        chunk = [[24, 25], [26, 27]]
    else:
        chunk = [[24, 25, 26], [27, 28, 29], [30, 31]]
    if final:
        return ctx + chunk
    out = ctx[:3]
    rest = ctx[3:]
    for i, cp in enumerate(chunk):
        out += [cp, rest[i]]
    out += rest[len(chunk) :]
    return out


def _build():
    nc = bacc.Bacc("TRN2", target_bir_lowering=False, debug=False)

    qdT = nc.dram_tensor("qdT", [HPD * HD, SEQ], F16, kind="ExternalInput")
    kdT = nc.dram_tensor("kdT", [HD, L], F16, kind="ExternalInput")
    vd = nc.dram_tensor("vd", [L, HD], F16, kind="ExternalInput")
    tri = nc.dram_tensor("tri", [128, 128], F32, kind="ExternalInput")
    # unnormalized oT rows per group (fp16: halves the output DMA; the
    # host divides in fp32)
    od = nc.dram_tensor("od", [HPD, NQC, 128, QCH], F16, kind="ExternalOutput")
    # raw fp16 denominator accumulators; host folds partitions + rows
    dacc_out = nc.dram_tensor(
        "dacc", [HPD, NQC, 128, NDACC, QCH], F16, kind="ExternalOutput"
    )

    with tile.TileContext(nc) as tc:
        with (
            tc.tile_pool(name="big", bufs=1) as big,
            tc.tile_pool(name="small", bufs=1) as small,
            tc.tile_pool(name="expp", bufs=8) as expp,
            tc.tile_pool(name="osb", bufs=2) as osb,
            tc.tile_pool(name="accsb", bufs=2) as accsb,
            tc.tile_pool(name="scps", bufs=2, space="PSUM") as scps,
            tc.tile_pool(name="accps", bufs=2, space="PSUM") as accps,
        ):
            # ---- constants ----
            tri_sb = small.tile([128, 128], F32, tag="tri")
            tri_h = small.tile([128, 128], F16, tag="tri_h")

            # ---- input loads (already fp16, no on-device casts) ----
            # the HWDGE descriptor engine is serial (~630ns/desc,
            # round-robin over the SP and ACT rings), so keep descriptors
            # few and consumption-ordered; v rides the gpsimd SWDGE whose
            # descriptor gen runs in parallel on the idle Pool engine.
            kT_full = big.tile([128, NT * 128], F16, name="kT", tag="kT")
            qAll = big.tile(
                [128, HPD, NQC, QCH], F16, name="qAll", tag="qAll"
            )
            v_c = [
                big.tile([128, NT // 4, HD], F16, name=f"v{i}", tag=f"v{i}")
                for i in range(4)
            ]
            vdr = vd.rearrange("(t p) d -> p t d", p=128)
            qdr = qdT.rearrange("(h p) (c q) -> p h c q", p=128, q=QCH)

            def k_dma(i):
                a, b = K_SEGS[i][0] * 128, K_SEGS[i][1] * 128
                nc.sync.dma_start(out=kT_full[:, a:b], in_=kdT[:, a:b])

            def v_dma(i):
                sl = slice(i * (NT // 4), (i + 1) * (NT // 4))
                nc.gpsimd.dma_start(out=v_c[i], in_=vdr[:, sl, :])

            k_dma(0)
            nc.scalar.dma_start(out=qAll[:, 0, 0, :], in_=qdr[:, 0, 0, :])
            nc.gpsimd.dma_start(out=v_c[0][:, 0:4, :], in_=vdr[:, 0:4, :])
            nc.gpsimd.dma_start(out=v_c[0][:, 4:8, :], in_=vdr[:, 4:8, :])
            k_dma(1)
            nc.scalar.dma_start(out=qAll[:, 0, 1, :], in_=qdr[:, 0, 1, :])
            v_dma(3)
            k_dma(2)
            nc.scalar.dma_start(out=tri_sb, in_=tri[:, :])
            nc.vector.tensor_copy(out=tri_h, in_=tri_sb)
            v_dma(1)
            k_dma(3)
            nc.scalar.dma_start(out=qAll[:, 1:, :, :], in_=qdr[:, 1:, :, :])
            v_dma(2)

            def kT_at(lt):
                off = K_TILE_POS[lt] * 128
                return kT_full[:, off : off + 128]

            def v_at(lt):
                return v_c[lt // 8][:, lt % 8, :]

            # ---- main attention: one flat lag-2 software pipeline over
            # all (head, q-chunk, l-slot-task) tasks. PV(p-2)'s inputs
            # (exp + mask mul) are always complete by emission, so the
            # in-order PE never blocks a later QK^T behind a PV waiting
            # on the ACT/DVE chain.
            tasks = []  # (h, c, [lt..], first, last)
            glist = []
            for h in range(HPD):
                for c in range(NQC):
                    prs = _group_tasks(
                        h, c, (h, c) == (HPD - 1, NQC - 1)
                    )
                    glist.append((h, c))
                    for pi, pr in enumerate(prs):
                        tasks.append(
                            (h, c, pr, pi == 0, pi == len(prs) - 1)
                        )
            prev_group = {
                g: glist[i - 1] for i, g in enumerate(glist) if i
            }
            task_index_in_group = []
            seen = {}
            for h, c, pr, first, last in tasks:
                seen[(h, c)] = 0 if first else seen[(h, c)] + 1
                task_index_in_group.append(seen[(h, c)])

            group_psum = {}  # (h, c) -> acc
            group_dacc = {}  # (h, c) -> dacc_all tile [128, NDACC, QCH]
            group_state = {}  # (h, c) -> [pending slots, ninit, rr]
            ex_tiles = [None] * len(tasks)

            def start_col(lt, c):
                """first unmasked q column for this l-tile (left of it the
                row block is fully masked; nothing is computed there)."""
                if lt < NT_CTX:
                    return 0
                b = lt - NT_CTX - 4 * c
                return max(b, 0) * 128

            def emit_qkt(p):
                h, c, pr, _, _ = tasks[p]
                qmv = qAll[:, h, c, :]
                sc = scps.tile([128, NSLOT, QCH], F32, tag="sc")
                ex = expp.tile([128, NSLOT, QCH], F16, tag="ex")
                ex_tiles[p] = ex
                for s, lt in enumerate(pr):
                    st = start_col(lt, c)
                    nc.tensor.matmul(
                        sc[:, s, st:],
                        kT_at(lt),
                        qmv[:, st:],
                        start=True,
                        stop=True,
                    )
                stp = min(start_col(lt, c) for lt in pr)
                ns = len(pr)
                nc.scalar.activation(
                    out=ex[:, 0:ns, stp:],
                    in_=sc[:, 0:ns, stp:],
                    func=mybir.ActivationFunctionType.Exp,
                    scale=SCALE,
                )
                # causal mask as a 0/1 multiply on the exp output: keeps
                # the DVE mask op off the ACT-critical chain (exp never
                # waits on it; the PV/denominator consumers run two tasks
                # behind). Columns left of the diagonal are never read.
                for s, lt in enumerate(pr):
                    b = lt - NT_CTX - 4 * c
                    if lt >= NT_CTX and 0 <= b <= 3:
                        st = start_col(lt, c)
                        nc.vector.tensor_mul(
                            out=ex[:, s, st : st + 128],
                            in0=ex[:, s, st : st + 128],
                            in1=tri_h,
                        )

            def emit_pv(p):
                h, c, pr, first, last = tasks[p]
                pi = task_index_in_group[p]
                if first:
                    group_psum[(h, c)] = accps.tile(
                        [128, QCH], F32, name="acc", tag="acc"
                    )
                    group_dacc[(h, c)] = accsb.tile(
                        [128, NDACC, QCH], F16, name="dacc", tag="dacc"
                    )
                    group_state[(h, c)] = [[], 0, 0]
                acc = group_psum[(h, c)]
                dacc = group_dacc[(h, c)]
                ex = ex_tiles[p]
                for s, lt in enumerate(pr):
                    st = start_col(lt, c)
                    is_first = first and s == 0
                    is_last = last and s == len(pr) - 1
                    nc.tensor.matmul(
                        acc[:, st:],
                        v_at(lt),
                        ex[:, s, st:],
                        start=is_first,
                        stop=is_last,
                    )
                # denominator accumulation on the DVE in fp16 (2x mode)
                # into NDACC interleaved rows; the first NDACC rows
                # initialize from pairs of full-width context slots.
                state = group_state[(h, c)]
                pending, ninit, rr = state
                for s, lt in enumerate(pr):
                    pending.append((ex, s, start_col(lt, c)))
                while pending:
                    if ninit < NDACC:
                        if len(pending) < 2:
                            break
                        (ea, sa, _), (eb, sb, _) = pending[0], pending[1]
                        del pending[:2]
                        nc.vector.tensor_add(
                            out=dacc[:, ninit, :],
                            in0=ea[:, sa, :],
                            in1=eb[:, sb, :],
                        )
                        ninit += 1
                    else:
                        ea, sa, st = pending.pop(0)
                        nc.vector.tensor_add(
                            out=dacc[:, rr, st:],
                            in0=dacc[:, rr, st:],
                            in1=ea[:, sa, st:],
                        )
                        rr = (rr + 1) % NDACC
                state[1], state[2] = ninit, rr
                # defer the previous group's epilogue a couple of tasks so
                # the in-order PE doesn't stall at the group boundary
                if pi == 2 and (h, c) != (0, 0):
                    emit_epilogue(*prev_group[(h, c)])

            def emit_epilogue(h, c):
                acc = group_psum[(h, c)]
                dacc = group_dacc[(h, c)]
                # ship unnormalized oT + raw denominator accumulators
                oT_sb = osb.tile([128, QCH], F16, tag="oT_sb")
                nc.vector.tensor_copy(out=oT_sb, in_=acc)
                nc.sync.dma_start(out=od[h, c, :, :], in_=oT_sb)
                nc.sync.dma_start(out=dacc_out[h, c, :, :, :], in_=dacc)

            for p in range(len(tasks) + 2):
                if p < len(tasks):
                    emit_qkt(p)
                if p >= 2:
                    emit_pv(p - 2)
            emit_epilogue(*glist[-1])
    nc.compile()
    return nc


def _prep_host(q, k, v, k_cache, v_cache, slot_mapping, context_slots):
    """Resolve the paged-cache scatter+gather on the host.

    Equivalent to: cache.at[slot_mapping].set(new); gather cache[context_slots];
    concat with the new chunk.
    """
    kh = np.ascontiguousarray(k).reshape(SEQ, NKVH, HD)
    vh = np.ascontiguousarray(v).reshape(SEQ, NKVH, HD)
    sm = np.asarray(slot_mapping)
    cs = np.asarray(context_slots)

    k_ctx = np.asarray(k_cache)[cs].copy()
    v_ctx = np.asarray(v_cache)[cs].copy()
    # overwrite any context slot that the new chunk was scattered into
    order = np.argsort(sm, kind="stable")
    ss = sm[order]
    j = np.searchsorted(ss, cs)
    jc = np.minimum(j, len(ss) - 1)
    hit = ss[jc] == cs
    if hit.any():
        src = order[jc[hit]]
        k_ctx[hit] = kh[src]
        v_ctx[hit] = vh[src]

    k_all = np.concatenate([k_ctx, kh], axis=0)  # [L, NKVH, HD]
    v_all = np.concatenate([v_ctx, vh], axis=0)
    return k_all, v_all


# results of the last run (exec time etc), for the local test harness
last_results = None


def kernel(q, k, v, k_cache, v_cache, slot_mapping, context_slots):
    global last_results
    q = np.asarray(q, dtype=np.float32)
    k_all, v_all = _prep_host(
        q, np.asarray(k), np.asarray(v), k_cache, v_cache, slot_mapping, context_slots
    )

    if "nc" not in _CACHE:
        _CACHE["nc"] = _build()
    nc = _CACHE["nc"]

    # 0/1 visibility mask for the diagonal blocks (applied multiplicatively
    # to the exp output on-device)
    tri = (
        np.arange(128)[None, :] >= np.arange(128)[:, None]
    ).astype(np.float32)

    # kdT l-columns permuted into the kernel's consumption order
    # [tiles 0..15 | 24..31 | 16..23] so each k DMA is one contiguous range
    kperm = np.concatenate(
        [np.arange(0, 2048), np.arange(3072, 4096), np.arange(2048, 3072)]
    )
    in_maps = []
    for d in range(NDEV):
        in_maps.append(
            {
                "qdT": np.ascontiguousarray(
                    q[:, d * HPD * HD : (d + 1) * HPD * HD].T
                ).astype(np.float16),
                "kdT": np.ascontiguousarray(
                    k_all[kperm, d, :].T
                ).astype(np.float16),
                "vd": np.ascontiguousarray(v_all[:, d, :]).astype(np.float16),
                "tri": tri,
            }
        )

    res = run_bass_kernel_spmd(nc, in_maps, core_ids=list(range(NDEV)))
    last_results = res

    out = np.empty((SEQ, NH * HD), dtype=np.float32)
    for d in range(NDEV):
        odp = res.results[d]["od"].reshape(HPD, NQC, 128, QCH)
        oT = odp.transpose(0, 2, 1, 3).reshape(HPD, HD, SEQ)
        # denominators: fold the partition axis + the NDACC rows in fp32
        da = res.results[d]["dacc"].reshape(HPD, NQC, 128, NDACC, QCH)
        sums = da.astype(np.float32).sum(axis=(2, 3)).reshape(HPD, SEQ)
        o = oT / sums[:, None, :]  # [HPD, HD, SEQ]
        out[:, d * HPD * HD : (d + 1) * HPD * HD] = (
            o.transpose(2, 0, 1).reshape(SEQ, HPD * HD)
        )
    return out


# revision 59
# speedup vs baseline: 1.0043x; 1.0043x over previous
"""Chunked-prefill paged attention kernel for Trainium2 (Bass/Tile), 8 cores.

Sharding: tensor-parallel over heads. Core i handles q heads 4i..4i+3 and
kv head i. The paged-cache scatter/gather (pure data movement, index-driven)
is resolved on the host; each core runs dense attention over the gathered
[ctx | chunk] keys/values for its kv head.

Per-core layout ("transposed scores"): q and k arrive pre-transposed and
pre-cast to fp16 from the host ([d, seq] / [d, L]):
  scoresT[l, q] = kT_tile (stationary) x qT (moving)     -> PSUM f32
  exp on the scalar engine (PSUM -> SBUF, fp16)
  oT[d, q]     += v_tile (stationary) x expT (moving)    -> PSUM f32
Tasks cover up to THREE 128-row l-tiles so each exp instruction amortizes
the fixed PSUM/SBUF access overhead over 1536 columns; the PSUM budget is
exactly 8 banks: score tiles [128,3,512] (3 banks) double-buffered + the
output accumulator (1 bank) double-buffered.

Softmax denominators: the DVE accumulates the fp16 exp tiles elementwise
(2-byte operands hit the DVE 2x mode) into 4 interleaved accumulator rows
of one [128, 4, 512] tile per group; the tile ships to the host as-is and
the host does the final partition-fold + divide (cheap numpy). The causal
mask is a 0/1 multiply on the exp output (DVE), off the ACT critical path.

fp16 operands stream the PE at 1 cycle/row (same as f32r) with no minimum
moving-width constraint, so causal trimming is exact at 128 granularity.
"""

import numpy as np

import concourse.bacc as bacc
import concourse.bass as bass
import concourse.mybir as mybir
import concourse.tile as tile
from concourse.bass_utils import run_bass_kernel_spmd

NH, NKVH, HD = 32, 8, 128
SCALE = 0.08838834764831845  # 1/sqrt(128)
SEQ, CTX = 1024, 3072
L = CTX + SEQ  # 4096
NDEV = 8
HPD = NH // NDEV  # q heads per device
QCH = 512  # q columns per group (psum bank width in f32)
NQC = SEQ // QCH  # q chunks
NDACC = 4  # interleaved fp16 denominator accumulator rows per group
NSLOT = 3  # l-tiles per task
NT = L // 128  # 32 l-tiles total
NT_CTX = CTX // 128  # 24 context l-tiles

F32 = mybir.dt.float32
F16 = mybir.dt.float16

# kdT arrives from the host with l-tiles permuted into consumption order
# [0..15 | 24..31 | 16..23], so every k DMA is a contiguous range. Four
# descriptors, sized so each arrives just before its first consumer.
K_SEGS = [(0, 3), (3, 9), (9, 24), (24, 32)]  # in sbuf tile positions
K_TILE_POS = {}  # dram l-tile -> sbuf tile position
for _lt in range(32):
    K_TILE_POS[_lt] = _lt if _lt < 16 else (_lt - 8 if _lt >= 24 else _lt + 8)

_CACHE = {}


def _group_tasks(h, c, final):
    """Slot lists for group (h, c): 8 context triples + chunk tasks.

    Context tiles fill triples; masked chunk tiles group together so the
    exp can skip their shared fully-masked left region. Chunk tasks sit
    mid-group (the DVE mask-mul rides behind PE work), except in the
    final group where they come last so the drain tail is the narrowest
    task.
    """
    ctx = [[3 * i, 3 * i + 1, 3 * i + 2] for i in range(NT_CTX // 3)]
    if c == 0:
        chunk = [[24, 25], [26, 27]]
    else:
        chunk = [[24, 25, 26], [27, 28, 29], [30, 31]]
    if final:
        return ctx + chunk
    out = ctx[:3]
    rest = ctx[3:]
    for i, cp in enumerate(chunk):
        out += [cp, rest[i]]
    out += rest[len(chunk) :]
    return out


def _build():
    nc = bacc.Bacc("TRN2", target_bir_lowering=False, debug=False)

    qdT = nc.dram_tensor("qdT", [HPD * HD, SEQ], F16, kind="ExternalInput")
    kdT = nc.dram_tensor("kdT", [HD, L], F16, kind="ExternalInput")
    vd = nc.dram_tensor("vd", [L, HD], F16, kind="ExternalInput")
    tri = nc.dram_tensor("tri", [128, 128], F32, kind="ExternalInput")
    # unnormalized oT rows per group (fp16: halves the output DMA; the
    # host divides in fp32). 4 spare columns carry the final group's
    # folded denominators so its tail needs only one DMA.
    od = nc.dram_tensor(
        "od", [HPD, NQC, 128, QCH + 4], F16, kind="ExternalOutput"
    )
    # raw fp16 denominator accumulators; host folds partitions + rows.
    # The final group instead folds on the PE (its 4KB dacc DMA would
    # dominate the drain) and ships a [128, NQSUB] result.
    dacc_out = nc.dram_tensor(
        "dacc", [HPD, NQC, 128, NDACC, QCH], F16, kind="ExternalOutput"
    )

    with tile.TileContext(nc) as tc:
        with (
            tc.tile_pool(name="big", bufs=1) as big,
            tc.tile_pool(name="small", bufs=1) as small,
            tc.tile_pool(name="expp", bufs=8) as expp,
            tc.tile_pool(name="osb", bufs=2) as osb,
            tc.tile_pool(name="accsb", bufs=2) as accsb,
            tc.tile_pool(name="scps", bufs=2, space="PSUM") as scps,
            tc.tile_pool(name="accps", bufs=2, space="PSUM") as accps,
        ):
            # ---- constants ----
            tri_sb = small.tile([128, 128], F32, tag="tri")
            tri_h = small.tile([128, 128], F16, tag="tri_h")
            ones_f = small.tile([128, 1], F32, tag="ones_f")
            nc.vector.memset(ones_f, 1.0)
            ones_h = small.tile([128, 1], F16, tag="ones_h")
            nc.vector.tensor_copy(out=ones_h, in_=ones_f)

            # ---- input loads (already fp16, no on-device casts) ----
            # the HWDGE descriptor engine is serial (~630ns/desc,
            # round-robin over the SP and ACT rings), so keep descriptors
            # few and consumption-ordered; v rides the gpsimd SWDGE whose
            # descriptor gen runs in parallel on the idle Pool engine.
            kT_full = big.tile([128, NT * 128], F16, name="kT", tag="kT")
            qAll = big.tile(
                [128, HPD, NQC, QCH], F16, name="qAll", tag="qAll"
            )
            v_c = [
                big.tile([128, NT // 4, HD], F16, name=f"v{i}", tag=f"v{i}")
                for i in range(4)
            ]
            vdr = vd.rearrange("(t p) d -> p t d", p=128)
            qdr = qdT.rearrange("(h p) (c q) -> p h c q", p=128, q=QCH)

            def k_dma(i):
                a, b = K_SEGS[i][0] * 128, K_SEGS[i][1] * 128
                nc.sync.dma_start(out=kT_full[:, a:b], in_=kdT[:, a:b])

            def v_dma(i):
                sl = slice(i * (NT // 4), (i + 1) * (NT // 4))
                nc.gpsimd.dma_start(out=v_c[i], in_=vdr[:, sl, :])

            k_dma(0)
            nc.scalar.dma_start(out=qAll[:, 0, 0, :], in_=qdr[:, 0, 0, :])
            nc.gpsimd.dma_start(out=v_c[0][:, 0:4, :], in_=vdr[:, 0:4, :])
            nc.gpsimd.dma_start(out=v_c[0][:, 4:8, :], in_=vdr[:, 4:8, :])
            k_dma(1)
            nc.scalar.dma_start(out=qAll[:, 0, 1, :], in_=qdr[:, 0, 1, :])
            v_dma(3)
            k_dma(2)
            nc.scalar.dma_start(out=tri_sb, in_=tri[:, :])
            nc.vector.tensor_copy(out=tri_h, in_=tri_sb)
            v_dma(1)
            k_dma(3)
            nc.scalar.dma_start(out=qAll[:, 1:, :, :], in_=qdr[:, 1:, :, :])
            v_dma(2)

            def kT_at(lt):
                off = K_TILE_POS[lt] * 128
                return kT_full[:, off : off + 128]

            def v_at(lt):
                return v_c[lt // 8][:, lt % 8, :]

            # ---- main attention: one flat lag-2 software pipeline over
            # all (head, q-chunk, l-slot-task) tasks. PV(p-2)'s inputs
            # (exp + mask mul) are always complete by emission, so the
            # in-order PE never blocks a later QK^T behind a PV waiting
            # on the ACT/DVE chain.
            tasks = []  # (h, c, [lt..], first, last)
            glist = []
            for h in range(HPD):
                for c in range(NQC):
                    prs = _group_tasks(
                        h, c, (h, c) == (HPD - 1, NQC - 1)
                    )
                    glist.append((h, c))
                    for pi, pr in enumerate(prs):
                        tasks.append(
                            (h, c, pr, pi == 0, pi == len(prs) - 1)
                        )
            prev_group = {
                g: glist[i - 1] for i, g in enumerate(glist) if i
            }
            task_index_in_group = []
            seen = {}
            for h, c, pr, first, last in tasks:
                seen[(h, c)] = 0 if first else seen[(h, c)] + 1
                task_index_in_group.append(seen[(h, c)])

            group_psum = {}  # (h, c) -> acc
            group_dacc = {}  # (h, c) -> dacc_all tile [128, NDACC, QCH]
            group_state = {}  # (h, c) -> [pending slots, ninit, rr]
            ex_tiles = [None] * len(tasks)

            def start_col(lt, c):
                """first unmasked q column for this l-tile (left of it the
                row block is fully masked; nothing is computed there)."""
                if lt < NT_CTX:
                    return 0
                b = lt - NT_CTX - 4 * c
                return max(b, 0) * 128

            def emit_qkt(p):
                h, c, pr, _, _ = tasks[p]
                qmv = qAll[:, h, c, :]
                sc = scps.tile([128, NSLOT, QCH], F32, tag="sc")
                ex = expp.tile([128, NSLOT, QCH], F16, tag="ex")
                ex_tiles[p] = ex
                for s, lt in enumerate(pr):
                    st = start_col(lt, c)
                    nc.tensor.matmul(
                        sc[:, s, st:],
                        kT_at(lt),
                        qmv[:, st:],
                        start=True,
                        stop=True,
                    )
                stp = min(start_col(lt, c) for lt in pr)
                ns = len(pr)
                nc.scalar.activation(
                    out=ex[:, 0:ns, stp:],
                    in_=sc[:, 0:ns, stp:],
                    func=mybir.ActivationFunctionType.Exp,
                    scale=SCALE,
                )
                # causal mask as a 0/1 multiply on the exp output: keeps
                # the DVE mask op off the ACT-critical chain (exp never
                # waits on it; the PV/denominator consumers run two tasks
                # behind). Columns left of the diagonal are never read.
                for s, lt in enumerate(pr):
                    b = lt - NT_CTX - 4 * c
                    if lt >= NT_CTX and 0 <= b <= 3:
                        st = start_col(lt, c)
                        nc.vector.tensor_mul(
                            out=ex[:, s, st : st + 128],
                            in0=ex[:, s, st : st + 128],
                            in1=tri_h,
                        )

            def emit_pv(p):
                h, c, pr, first, last = tasks[p]
                pi = task_index_in_group[p]
                if first:
                    group_psum[(h, c)] = accps.tile(
                        [128, QCH], F32, name="acc", tag="acc"
                    )
                    group_dacc[(h, c)] = accsb.tile(
                        [128, NDACC, QCH], F16, name="dacc", tag="dacc"
                    )
                    group_state[(h, c)] = [[], 0, 0]
                acc = group_psum[(h, c)]
                dacc = group_dacc[(h, c)]
                ex = ex_tiles[p]
                for s, lt in enumerate(pr):
                    st = start_col(lt, c)
                    is_first = first and s == 0
                    is_last = last and s == len(pr) - 1
                    nc.tensor.matmul(
                        acc[:, st:],
                        v_at(lt),
                        ex[:, s, st:],
                        start=is_first,
                        stop=is_last,
                    )
                # denominator accumulation on the DVE in fp16 (2x mode)
                # into NDACC interleaved rows; the first NDACC rows
                # initialize from pairs of full-width context slots.
                state = group_state[(h, c)]
                pending, ninit, rr = state
                for s, lt in enumerate(pr):
                    pending.append((ex, s, start_col(lt, c)))
                while pending:
                    if ninit < NDACC:
                        if len(pending) < 2:
                            break
                        (ea, sa, _), (eb, sb, _) = pending[0], pending[1]
                        del pending[:2]
                        nc.vector.tensor_add(
                            out=dacc[:, ninit, :],
                            in0=ea[:, sa, :],
                            in1=eb[:, sb, :],
                        )
                        ninit += 1
                    else:
                        ea, sa, st = pending.pop(0)
                        nc.vector.tensor_add(
                            out=dacc[:, rr, st:],
                            in0=dacc[:, rr, st:],
                            in1=ea[:, sa, st:],
                        )
                        rr = (rr + 1) % NDACC
                state[1], state[2] = ninit, rr
                # defer the previous group's epilogue a couple of tasks so
                # the in-order PE doesn't stall at the group boundary
                if pi == 2 and (h, c) != (0, 0):
                    emit_epilogue(*prev_group[(h, c)])

            def emit_epilogue(h, c):
                acc = group_psum[(h, c)]
                dacc = group_dacc[(h, c)]
                # ship unnormalized oT + raw denominator accumulators
                oT_sb = osb.tile([128, QCH + 4], F16, tag="oT_sb")
                nc.vector.tensor_copy(out=oT_sb[:, 0:QCH], in_=acc)
                if (h, c) != glist[-1]:
                    nc.sync.dma_start(
                        out=od[h, c, :, 0:QCH], in_=oT_sb[:, 0:QCH]
                    )
                    nc.sync.dma_start(
                        out=dacc_out[h, c, :, :, :], in_=dacc
                    )
                    return
                # final group: partition-fold on the PE (all QK^T work is
                # done, so an sc-pool tile is free to recycle) and ship a
                # tiny folded result instead of the 4KB dacc tile:
                # fold[q, j] = sum_a sum_l dacc[l, a, j*128 + q]
                nsub = QCH // 128
                fold = scps.tile([128, NSLOT, QCH], F32, tag="sc")
                for j in range(nsub):
                    for a in range(NDACC):
                        nc.tensor.matmul(
                            fold[:, 0, j : j + 1],
                            dacc[:, a, j * 128 : (j + 1) * 128],
                            ones_h,
                            start=(a == 0),
                            stop=(a == NDACC - 1),
                        )
                nc.vector.tensor_copy(
                    out=oT_sb[:, QCH:], in_=fold[:, 0, 0:nsub]
                )
                nc.sync.dma_start(out=od[h, c, :, :], in_=oT_sb)

            for p in range(len(tasks) + 2):
                if p < len(tasks):
                    emit_qkt(p)
                if p >= 2:
                    emit_pv(p - 2)
            emit_epilogue(*glist[-1])
    nc.compile()
    return nc


def _prep_host(q, k, v, k_cache, v_cache, slot_mapping, context_slots):
    """Resolve the paged-cache scatter+gather on the host.

    Equivalent to: cache.at[slot_mapping].set(new); gather cache[context_slots];
    concat with the new chunk.
    """
    kh = np.ascontiguousarray(k).reshape(SEQ, NKVH, HD)
    vh = np.ascontiguousarray(v).reshape(SEQ, NKVH, HD)
    sm = np.asarray(slot_mapping)
    cs = np.asarray(context_slots)

    k_ctx = np.asarray(k_cache)[cs].copy()
    v_ctx = np.asarray(v_cache)[cs].copy()
    # overwrite any context slot that the new chunk was scattered into
    order = np.argsort(sm, kind="stable")
    ss = sm[order]
    j = np.searchsorted(ss, cs)
    jc = np.minimum(j, len(ss) - 1)
    hit = ss[jc] == cs
    if hit.any():
        src = order[jc[hit]]
        k_ctx[hit] = kh[src]
        v_ctx[hit] = vh[src]

    k_all = np.concatenate([k_ctx, kh], axis=0)  # [L, NKVH, HD]
    v_all = np.concatenate([v_ctx, vh], axis=0)
    return k_all, v_all


# results of the last run (exec time etc), for the local test harness
last_results = None


def kernel(q, k, v, k_cache, v_cache, slot_mapping, context_slots):
    global last_results
    q = np.asarray(q, dtype=np.float32)
    k_all, v_all = _prep_host(
        q, np.asarray(k), np.asarray(v), k_cache, v_cache, slot_mapping, context_slots
    )

    if "nc" not in _CACHE:
        _CACHE["nc"] = _build()
    nc = _CACHE["nc"]

    # 0/1 visibility mask for the diagonal blocks (applied multiplicatively
    # to the exp output on-device)
    tri = (
        np.arange(128)[None, :] >= np.arange(128)[:, None]
    ).astype(np.float32)

    # kdT l-columns permuted into the kernel's consumption order
    # [tiles 0..15 | 24..31 | 16..23] so each k DMA is one contiguous range
    kperm = np.concatenate(
        [np.arange(0, 2048), np.arange(3072, 4096), np.arange(2048, 3072)]
    )
    in_maps = []
    for d in range(NDEV):
        in_maps.append(
            {
                "qdT": np.ascontiguousarray(
                    q[:, d * HPD * HD : (d + 1) * HPD * HD].T
                ).astype(np.float16),
                "kdT": np.ascontiguousarray(
                    k_all[kperm, d, :].T
                ).astype(np.float16),
                "vd": np.ascontiguousarray(v_all[:, d, :]).astype(np.float16),
                "tri": tri,
            }
        )

    res = run_bass_kernel_spmd(nc, in_maps, core_ids=list(range(NDEV)))
    last_results = res

    out = np.empty((SEQ, NH * HD), dtype=np.float32)
    for d in range(NDEV):
        odp = res.results[d]["od"].reshape(HPD, NQC, 128, QCH + 4)
        oT = odp[:, :, :, :QCH].transpose(0, 2, 1, 3).reshape(HPD, HD, SEQ)
        # denominators: fold the partition axis + the NDACC rows in fp32;
        # the final group ships pre-folded as sums_fin[p, j] with
        # q = j*128 + p (within its 512-column block)
        da = res.results[d]["dacc"].reshape(HPD, NQC, 128, NDACC, QCH)
        sums = da.astype(np.float32).sum(axis=(2, 3)).reshape(HPD, SEQ)
        sf = odp[HPD - 1, NQC - 1, :, QCH:].astype(np.float32)
        sums[HPD - 1, (NQC - 1) * QCH :] = sf.T.reshape(QCH)
        o = oT / sums[:, None, :]  # [HPD, HD, SEQ]
        out[:, d * HPD * HD : (d + 1) * HPD * HD] = (
            o.transpose(2, 0, 1).reshape(SEQ, HPD * HD)
        )
    return out
